# revision 21
# baseline (speedup 1.0000x reference)
"""Trainium2 Bass kernel for DirCFConv-style GNN message passing, v2.

Computes, for inputs s:(B,N,H) f32, ef_mask:(B,N,N,H) f32, W:(H,H), b:(H,):
    m   = SiLU(LayerNorm(s @ W.T + b))          # (B,N,H)
    out[b,i,h] = sum_j ef_mask[b,i,j,h] * m[b,j,h]

Sharding: 8 cores, core c handles batch b = c // 2 and query-node half
i in [ (c%2)*256, (c%2)*256+256 ).  Each core streams its 64 MiB mask
shard from HBM.

v2 layout: SBUF partition p holds the JJ=4 consecutive j's {4p..4p+3},
so each DMA descriptor covers a contiguous (jj,h) run of 2 KiB (the
old j-major layout forced 512 B descriptors, which capped the 16 SDMA
engines at ~270 GB/s and made descriptor count the bottleneck).  The
multiply uses a partition-permuted m (m_perm[p,jj,:] = m[4p+jj,:])
broadcast along i with a 0-stride AP, and the j-reduction is JJ
accumulating PE matmuls per query node i on a bf16 product (bf16
stationary = single-pass LDWEIGHTS; fp32 was 2 passes at 4x row cost
and made the PE the bottleneck at 446us).
"""

import numpy as np

import concourse.bass as bass
import concourse.bacc as bacc
import concourse.tile as tile
from concourse import mybir
from concourse.bass_utils import run_bass_kernel_spmd
from concourse.masks import make_identity

B, N, H = 4, 512, 128
P = 128
JJ = N // P           # 4 consecutive j's per partition
ISUB = 16             # i's per mask tile -> 4 MiB DMAs
IH = N // 2           # 256 i's per core
N_CORES = 8
LN_EPS = 1e-5
F32 = mybir.dt.float32
BF16 = mybir.dt.bfloat16


def build_nc(ih=IH, prod_dtype=BF16, repeat=1, do_mul=True, do_mm=True):
    nc = bacc.Bacc()
    s_d = nc.declare_dram_parameter("s", [N, H], F32, isOutput=False)
    w_d = nc.declare_dram_parameter("w", [H, H], F32, isOutput=False)
    b_d = nc.declare_dram_parameter("b", [H], F32, isOutput=False)
    mask_d = nc.declare_dram_parameter("mask", [ih, N, H], F32, isOutput=False)
    out_d = nc.declare_dram_parameter("out", [ih, H], F32, isOutput=True)

    nit = ih // ISUB
    with tile.TileContext(nc) as tc:
        with (
            tc.tile_pool(name="consts", bufs=1) as consts,
            tc.tile_pool(name="small", bufs=4) as small,
            tc.tile_pool(name="loads", bufs=4) as loads,
            tc.tile_pool(name="prods", bufs=3) as prods,
            tc.tile_pool(name="outs", bufs=3) as outs,
        ):
            stage1_psum = tc.tile_pool(name="spsum", bufs=1, space="PSUM")
            spsum = stage1_psum.__enter__()
            # ---------------- constants ----------------
            # All constants are produced on gpsimd BEFORE make_identity so the
            # single carrier wait (Pool sem) covers every one of them.
            ones_col = consts.tile([P, 1], BF16 if do_mul else F32)
            nc.gpsimd.memset(ones_col, 1.0)
            ones_row = consts.tile([1, P], F32)
            nc.gpsimd.memset(ones_row, 1.0)
            # eps on DVE: its consumer (ACT Sqrt) already waits on DVE for mv,
            # and one DVE sem wait covers both (Activation also allows only 1).
            eps_t = consts.tile([P, 1], F32)
            nc.vector.memset(eps_t, LN_EPS)
            ident = consts.tile([P, P], F32)
            make_identity(nc, ident)

            w_sb = consts.tile([H, H], F32)
            nc.scalar.dma_start(out=w_sb, in_=w_d[:, :])
            bias_sb = consts.tile([1, H], F32)
            b_ap = b_d[:]
            bias_src = bass.AP(
                tensor=b_ap.tensor, offset=b_ap.offset, ap=[[0, 1]] + list(b_ap.ap)
            )
            nc.scalar.dma_start(out=bias_sb, in_=bias_src)

            # Wait-carrier: walrus allows only ONE sync wait per Matmult, so
            # absorb the gpsimd(identity) dependency into a throwaway PE op;
            # later matmuls then only carry their own single DMA/engine wait.
            carrier_ps = spsum.tile([P, P], F32)
            nc.tensor.transpose(carrier_ps, ident, ident)

            # W^T via PE-transpose: (o,h) -> (h,o)
            wT_ps = spsum.tile([H, H], F32)
            nc.tensor.transpose(wT_ps, w_sb, ident)
            wT_sb = consts.tile([H, H], F32)
            nc.scalar.copy(wT_sb, wT_ps)

            # ------------- m = SiLU(LN(s @ W.T + b)) -------------
            # Row block jj holds rows {j : j mod JJ == jj} so partition p of
            # block jj is node j = JJ*p + jj -- i.e. m lands directly in the
            # m_perm[p, jj, :] layout stage 2 needs.  LN/SiLU are row-wise, so
            # any 128-row grouping is valid; only the s gather AP changes.
            sT_all = spsum.tile([P, JJ * P], F32)
            h_all = spsum.tile([P, JJ * H], F32)
            s_full = s_d[:, :]
            s_sbs = []
            for jj in range(JJ):
                s_sb = small.tile([P, H], F32, tag=f"s_sb{jj}")
                s_src = bass.AP(
                    tensor=s_full.tensor,
                    offset=jj * H,
                    ap=[[JJ * H, P], [1, H]],
                )
                nc.scalar.dma_start(out=s_sb, in_=s_src)
                s_sbs.append(s_sb)
                nc.tensor.matmul(
                    sT_all[:, jj * P:(jj + 1) * P],
                    lhsT=s_sb,
                    rhs=ident,
                    is_transpose=True,
                    start=(jj == 0),
                    stop=(jj == JJ - 1),
                )
            sT_sb = consts.tile([P, JJ * P], F32)
            nc.scalar.copy(sT_sb, sT_all)
            for jj in range(JJ):
                nc.tensor.matmul(
                    h_all[:, jj * H:(jj + 1) * H],
                    lhsT=sT_sb[:, jj * P:(jj + 1) * P],
                    rhs=wT_sb,
                    start=(jj == 0),
                    stop=False,
                )
                nc.tensor.matmul(
                    h_all[:, jj * H:(jj + 1) * H],
                    lhsT=ones_row,
                    rhs=bias_sb,
                    start=False,
                    stop=(jj == JJ - 1),
                )

            # m_perm[p, jj, :] = m[JJ*p + jj, :]
            m_perm = consts.tile([P, JJ, H], F32)
            for jj in range(JJ):
                h_ps = h_all[:, jj * H:(jj + 1) * H]
                stats = small.tile([P, 6], F32)
                nc.vector.bn_stats(stats, h_ps)
                mv = small.tile([P, 2], F32)
                nc.vector.bn_aggr(mv, stats)
                xc = small.tile([P, H], F32)
                nc.vector.tensor_scalar_sub(xc, h_ps, mv[:, 0:1])
                stdv = small.tile([P, 1], F32)
                nc.scalar.activation(
                    stdv, mv[:, 1:2], mybir.ActivationFunctionType.Sqrt, bias=eps_t
                )
                rstd = small.tile([P, 1], F32)
                nc.vector.reciprocal(rstd, stdv)
                xn = small.tile([P, H], F32)
                nc.vector.tensor_scalar_mul(xn, xc, rstd)
                sg = small.tile([P, H], F32)
                nc.scalar.activation(sg, xn, mybir.ActivationFunctionType.Sigmoid)
                nc.vector.tensor_mul(m_perm[:, jj, :], xn, sg)

            # m broadcast along the i axis: 0-stride free axis, no replication.
            def m_bcast(n):
                return bass.AP(
                    tensor=m_perm.tensor,
                    offset=m_perm.offset,
                    ap=[list(m_perm.ap[0]), [0, n]]
                    + [list(x) for x in m_perm.ap[1:]],
                )

            # stage-1 PSUM pools stay open: releasing them would put a
            # (PE+DVE) release-wait on stage-2's first Matmult, which walrus
            # cannot encode.
            # ------------- out[i,h] = sum_j mask[i,j,h] * m[j,h] -------------
            # acc2[h, i] += pt[:, ii, jj, :].T @ ones  (partition-reduce over
            # p via PE, free-axis reduce over jj via PSUM accumulation).
            opsum_cm = tc.tile_pool(name="opsum", bufs=1, space="PSUM")
            opsum = opsum_cm.__enter__()
            tpsum_cm = tc.tile_pool(name="tpsum", bufs=2, space="PSUM")
            tpsum = tpsum_cm.__enter__()
            acc2 = opsum.tile([P, ih], F32)
            for rp in range(repeat):
              for it in range(nit):
                mt = loads.tile([P, ISUB, JJ, H], F32, tag="mt", name=f"mt{it}")
                # The LAST FOUR tiles' DMAs are split into 1 MiB quarters with
                # their own completion sems.  The two HWDGE queues interleave
                # at packet granularity, so whole 4 MiB tiles complete in
                # near-simultaneous pairs and the DVE accumulates an ~17us
                # multiply backlog by stream end; 2.3us quarter-multiplies
                # chasing 1 MiB arrivals drain that backlog so the tail after
                # the last byte is one quarter-multiply, not two tiles' worth.
                last = it >= nit - 5 and nit > 5 and do_mul and do_mm
                nsub = 4 if last else 1
                sub = ISUB // nsub
                for q in range(nsub):
                    src = mask_d[
                        it * ISUB + q * sub:it * ISUB + (q + 1) * sub, :, :
                    ].rearrange("i (p jj) h -> p i jj h", jj=JJ)
                    # Alternate the two HWDGE queues (SP / Activation) so
                    # descriptor-gen + completion latency of one queue hides
                    # behind the other's transfers.
                    deng = nc.sync if (it + q) % 2 == 0 else nc.scalar
                    deng.dma_start(out=mt[:, q * sub:(q + 1) * sub], in_=src)
                    if do_mul:
                        pt = prods.tile([P, sub, JJ, H], BF16, tag=f"pt{sub}",
                                        name=f"pt{it}_{q}",
                                        bufs=3 if sub == ISUB else 2)
                        nc.vector.tensor_mul(
                            pt, mt[:, q * sub:(q + 1) * sub], m_bcast(sub)
                        )
                    else:
                        pt = mt[:, q * sub:(q + 1) * sub]
                    for ii in range(sub if do_mm else 0):
                        i = it * ISUB + q * sub + ii
                        for jj in range(JJ):
                            # One accumulation group spans the whole bank:
                            # start zeroes the full zero region, so only the
                            # global first/last matmuls carry start/stop.
                            nc.tensor.matmul(
                                acc2[:, i:i + 1],
                                lhsT=pt[:, ii, jj, :],
                                rhs=ones_col,
                                start=(it == 0 and q == 0 and ii == 0
                                       and jj == 0),
                                stop=(it == nit - 1 and q == nsub - 1
                                      and ii == sub - 1 and jj == JJ - 1),
                            )
            # epilogue: acc2 is [h, i]; transpose 128-blocks back to [i, h]
            accT = outs.tile([P, ih], F32, bufs=1)
            if do_mm:
                nc.vector.tensor_copy(accT, acc2)
            else:
                nc.vector.memset(accT, 0.0)
                nc.vector.tensor_copy(acc2[:, 0:1], accT[:, 0:1])
            for blk in range(ih // P):
                tp = tpsum.tile([P, P], F32, tag="tp", name=f"tp{blk}")
                nc.tensor.transpose(tp, accT[:, blk * P:(blk + 1) * P], ident)
                oT = outs.tile([P, P], F32, tag="oT", name=f"oT{blk}", bufs=2)
                nc.scalar.copy(oT, tp)
                deng = nc.sync if blk % 2 == 0 else nc.scalar
                deng.dma_start(out=out_d[blk * P:(blk + 1) * P, :], in_=oT)
            tpsum_cm.__exit__(None, None, None)
            opsum_cm.__exit__(None, None, None)
            stage1_psum.__exit__(None, None, None)
    nc.finalize()
    return nc


_NC_CACHE = {}


def _get_nc():
    key = "main"
    if key not in _NC_CACHE:
        _NC_CACHE[key] = build_nc()
    return _NC_CACHE[key]


def kernel(s, ef_mask, W, b):
    s = np.ascontiguousarray(s, dtype=np.float32)
    ef_mask = np.ascontiguousarray(ef_mask, dtype=np.float32)
    W = np.ascontiguousarray(W, dtype=np.float32)
    b = np.ascontiguousarray(b, dtype=np.float32)

    nc = _get_nc()
    in_maps = []
    for c in range(N_CORES):
        bb = c // 2
        half = c % 2
        in_maps.append(
            {
                "s": s[bb],
                "w": W,
                "b": b,
                "mask": ef_mask[bb, half * IH:(half + 1) * IH],
            }
        )
    res = run_bass_kernel_spmd(nc, in_maps, list(range(N_CORES))).results
    out = np.empty((B, N, H), dtype=np.float32)
    for c in range(N_CORES):
        bb = c // 2
        half = c % 2
        out[bb, half * IH:(half + 1) * IH] = res[c]["out"]
    return out


# revision 22
# speedup vs baseline: 1.0125x; 1.0125x over previous
"""Trainium2 Bass kernel for DirCFConv-style GNN message passing, v2.

Computes, for inputs s:(B,N,H) f32, ef_mask:(B,N,N,H) f32, W:(H,H), b:(H,):
    m   = SiLU(LayerNorm(s @ W.T + b))          # (B,N,H)
    out[b,i,h] = sum_j ef_mask[b,i,j,h] * m[b,j,h]

Sharding: 8 cores, core c handles batch b = c // 2 and query-node half
i in [ (c%2)*256, (c%2)*256+256 ).  Each core streams its 64 MiB mask
shard from HBM.

v2 layout: SBUF partition p holds the JJ=4 consecutive j's {4p..4p+3},
so each DMA descriptor covers a contiguous (jj,h) run of 2 KiB (the
old j-major layout forced 512 B descriptors, which capped the 16 SDMA
engines at ~270 GB/s and made descriptor count the bottleneck).  The
multiply uses a partition-permuted m (m_perm[p,jj,:] = m[4p+jj,:])
broadcast along i with a 0-stride AP, and the j-reduction is JJ
accumulating PE matmuls per query node i on a bf16 product (bf16
stationary = single-pass LDWEIGHTS; fp32 was 2 passes at 4x row cost
and made the PE the bottleneck at 446us).
"""

import numpy as np

import concourse.bass as bass
import concourse.bacc as bacc
import concourse.tile as tile
from concourse import mybir
from concourse.bass_utils import run_bass_kernel_spmd
from concourse.masks import make_identity

B, N, H = 4, 512, 128
P = 128
JJ = N // P           # 4 consecutive j's per partition
ISUB = 16             # i's per mask tile -> 4 MiB DMAs
IH = N // 2           # 256 i's per core
N_CORES = 8
LN_EPS = 1e-5
F32 = mybir.dt.float32
BF16 = mybir.dt.bfloat16


def build_nc(ih=IH, prod_dtype=BF16, repeat=1, do_mul=True, do_mm=True):
    nc = bacc.Bacc()
    s_d = nc.declare_dram_parameter("s", [N, H], F32, isOutput=False)
    w_d = nc.declare_dram_parameter("w", [H, H], F32, isOutput=False)
    b_d = nc.declare_dram_parameter("b", [H], F32, isOutput=False)
    mask_d = nc.declare_dram_parameter("mask", [ih, N, H], F32, isOutput=False)
    out_d = nc.declare_dram_parameter("out", [ih, H], F32, isOutput=True)

    nit = ih // ISUB
    with tile.TileContext(nc) as tc:
        with (
            tc.tile_pool(name="consts", bufs=1) as consts,
            tc.tile_pool(name="small", bufs=4) as small,
            tc.tile_pool(name="loads", bufs=4) as loads,
            tc.tile_pool(name="prods", bufs=3) as prods,
            tc.tile_pool(name="outs", bufs=3) as outs,
        ):
            stage1_psum = tc.tile_pool(name="spsum", bufs=1, space="PSUM")
            spsum = stage1_psum.__enter__()
            # ---------------- constants ----------------
            # All constants are produced on gpsimd BEFORE make_identity so the
            # single carrier wait (Pool sem) covers every one of them.
            ones_col = consts.tile([P, 1], BF16 if do_mul else F32)
            nc.gpsimd.memset(ones_col, 1.0)
            ones_row = consts.tile([1, P], F32)
            nc.gpsimd.memset(ones_row, 1.0)
            # eps on DVE: its consumer (ACT Sqrt) already waits on DVE for mv,
            # and one DVE sem wait covers both (Activation also allows only 1).
            eps_t = consts.tile([P, 1], F32)
            nc.vector.memset(eps_t, LN_EPS)
            ident = consts.tile([P, P], F32)
            make_identity(nc, ident)

            w_sb = consts.tile([H, H], F32)
            nc.scalar.dma_start(out=w_sb, in_=w_d[:, :])
            bias_sb = consts.tile([1, H], F32)
            b_ap = b_d[:]
            bias_src = bass.AP(
                tensor=b_ap.tensor, offset=b_ap.offset, ap=[[0, 1]] + list(b_ap.ap)
            )
            nc.scalar.dma_start(out=bias_sb, in_=bias_src)

            # Wait-carrier: walrus allows only ONE sync wait per Matmult, so
            # absorb the gpsimd(identity) dependency into a throwaway PE op;
            # later matmuls then only carry their own single DMA/engine wait.
            carrier_ps = spsum.tile([P, P], F32)
            nc.tensor.transpose(carrier_ps, ident, ident)

            # W^T via PE-transpose: (o,h) -> (h,o)
            wT_ps = spsum.tile([H, H], F32)
            nc.tensor.transpose(wT_ps, w_sb, ident)
            wT_sb = consts.tile([H, H], F32)
            nc.scalar.copy(wT_sb, wT_ps)

            # ------------- m = SiLU(LN(s @ W.T + b)) -------------
            # Row block jj holds rows {j : j mod JJ == jj} so partition p of
            # block jj is node j = JJ*p + jj -- i.e. m lands directly in the
            # m_perm[p, jj, :] layout stage 2 needs.  LN/SiLU are row-wise, so
            # any 128-row grouping is valid; only the s gather AP changes.
            sT_all = spsum.tile([P, JJ * P], F32)
            h_all = spsum.tile([P, JJ * H], F32)
            s_full = s_d[:, :]
            s_sbs = []
            for jj in range(JJ):
                s_sb = small.tile([P, H], F32, tag=f"s_sb{jj}")
                s_src = bass.AP(
                    tensor=s_full.tensor,
                    offset=jj * H,
                    ap=[[JJ * H, P], [1, H]],
                )
                nc.scalar.dma_start(out=s_sb, in_=s_src)
                s_sbs.append(s_sb)
                nc.tensor.matmul(
                    sT_all[:, jj * P:(jj + 1) * P],
                    lhsT=s_sb,
                    rhs=ident,
                    is_transpose=True,
                    start=(jj == 0),
                    stop=(jj == JJ - 1),
                )
            sT_sb = consts.tile([P, JJ * P], F32)
            nc.scalar.copy(sT_sb, sT_all)
            for jj in range(JJ):
                nc.tensor.matmul(
                    h_all[:, jj * H:(jj + 1) * H],
                    lhsT=sT_sb[:, jj * P:(jj + 1) * P],
                    rhs=wT_sb,
                    start=(jj == 0),
                    stop=False,
                )
                nc.tensor.matmul(
                    h_all[:, jj * H:(jj + 1) * H],
                    lhsT=ones_row,
                    rhs=bias_sb,
                    start=False,
                    stop=(jj == JJ - 1),
                )

            # m_perm[p, jj, :] = m[JJ*p + jj, :]
            m_perm = consts.tile([P, JJ, H], F32)
            for jj in range(JJ):
                h_ps = h_all[:, jj * H:(jj + 1) * H]
                stats = small.tile([P, 6], F32)
                nc.vector.bn_stats(stats, h_ps)
                mv = small.tile([P, 2], F32)
                nc.vector.bn_aggr(mv, stats)
                xc = small.tile([P, H], F32)
                nc.vector.tensor_scalar_sub(xc, h_ps, mv[:, 0:1])
                stdv = small.tile([P, 1], F32)
                nc.scalar.activation(
                    stdv, mv[:, 1:2], mybir.ActivationFunctionType.Sqrt, bias=eps_t
                )
                rstd = small.tile([P, 1], F32)
                nc.vector.reciprocal(rstd, stdv)
                xn = small.tile([P, H], F32)
                nc.vector.tensor_scalar_mul(xn, xc, rstd)
                sg = small.tile([P, H], F32)
                nc.scalar.activation(sg, xn, mybir.ActivationFunctionType.Sigmoid)
                nc.vector.tensor_mul(m_perm[:, jj, :], xn, sg)

            # m broadcast along the i axis: 0-stride free axis, no replication.
            def m_bcast(n):
                return bass.AP(
                    tensor=m_perm.tensor,
                    offset=m_perm.offset,
                    ap=[list(m_perm.ap[0]), [0, n]]
                    + [list(x) for x in m_perm.ap[1:]],
                )

            # stage-1 PSUM pools stay open: releasing them would put a
            # (PE+DVE) release-wait on stage-2's first Matmult, which walrus
            # cannot encode.
            # ------------- out[i,h] = sum_j mask[i,j,h] * m[j,h] -------------
            # acc2[h, i] += pt[:, ii, jj, :].T @ ones  (partition-reduce over
            # p via PE, free-axis reduce over jj via PSUM accumulation).
            opsum_cm = tc.tile_pool(name="opsum", bufs=1, space="PSUM")
            opsum = opsum_cm.__enter__()
            tpsum_cm = tc.tile_pool(name="tpsum", bufs=2, space="PSUM")
            tpsum = tpsum_cm.__enter__()
            acc2 = opsum.tile([P, ih], F32)
            for rp in range(repeat):
              for it in range(nit):
                mt = loads.tile([P, ISUB, JJ, H], F32, tag="mt", name=f"mt{it}")
                # The LAST FOUR tiles' DMAs are split into 1 MiB quarters with
                # their own completion sems.  The two HWDGE queues interleave
                # at packet granularity, so whole 4 MiB tiles complete in
                # near-simultaneous pairs and the DVE accumulates an ~17us
                # multiply backlog by stream end; 2.3us quarter-multiplies
                # chasing 1 MiB arrivals drain that backlog so the tail after
                # the last byte is one quarter-multiply, not two tiles' worth.
                # Every tile's DMA is issued as two 2 MiB halves on opposite
                # HWDGE queues: tiles then complete sequentially (~11.6us
                # apart) instead of in near-simultaneous pairs, which keeps
                # the DVE multiply backlog near zero through the stream.  The
                # final tile uses 1 MiB quarters so the post-stream tail is a
                # single 2.3us quarter-multiply.
                last = it == nit - 1 and nit > 1 and do_mul and do_mm
                nsub = 4 if last else (2 if nit > 1 and do_mul and do_mm else 1)
                sub = ISUB // nsub
                for q in range(nsub):
                    src = mask_d[
                        it * ISUB + q * sub:it * ISUB + (q + 1) * sub, :, :
                    ].rearrange("i (p jj) h -> p i jj h", jj=JJ)
                    # Alternate the two HWDGE queues (SP / Activation) so
                    # descriptor-gen + completion latency of one queue hides
                    # behind the other's transfers.
                    deng = nc.sync if (it + q) % 2 == 0 else nc.scalar
                    deng.dma_start(out=mt[:, q * sub:(q + 1) * sub], in_=src)
                    if do_mul:
                        pt = prods.tile([P, sub, JJ, H], BF16, tag=f"pt{sub}",
                                        name=f"pt{it}_{q}",
                                        bufs=2)
                        nc.vector.tensor_mul(
                            pt, mt[:, q * sub:(q + 1) * sub], m_bcast(sub)
                        )
                    else:
                        pt = mt[:, q * sub:(q + 1) * sub]
                    for ii in range(sub if do_mm else 0):
                        i = it * ISUB + q * sub + ii
                        for jj in range(JJ):
                            # One accumulation group spans the whole bank:
                            # start zeroes the full zero region, so only the
                            # global first/last matmuls carry start/stop.
                            nc.tensor.matmul(
                                acc2[:, i:i + 1],
                                lhsT=pt[:, ii, jj, :],
                                rhs=ones_col,
                                start=(it == 0 and q == 0 and ii == 0
                                       and jj == 0),
                                stop=(it == nit - 1 and q == nsub - 1
                                      and ii == sub - 1 and jj == JJ - 1),
                            )
            # epilogue: acc2 is [h, i]; transpose 128-blocks back to [i, h]
            accT = outs.tile([P, ih], F32, bufs=1)
            if do_mm:
                nc.vector.tensor_copy(accT, acc2)
            else:
                nc.vector.memset(accT, 0.0)
                nc.vector.tensor_copy(acc2[:, 0:1], accT[:, 0:1])
            for blk in range(ih // P):
                tp = tpsum.tile([P, P], F32, tag="tp", name=f"tp{blk}")
                nc.tensor.transpose(tp, accT[:, blk * P:(blk + 1) * P], ident)
                oT = outs.tile([P, P], F32, tag="oT", name=f"oT{blk}", bufs=2)
                nc.scalar.copy(oT, tp)
                deng = nc.sync if blk % 2 == 0 else nc.scalar
                deng.dma_start(out=out_d[blk * P:(blk + 1) * P, :], in_=oT)
            tpsum_cm.__exit__(None, None, None)
            opsum_cm.__exit__(None, None, None)
            stage1_psum.__exit__(None, None, None)
    nc.finalize()
    return nc


_NC_CACHE = {}


def _get_nc():
    key = "main"
    if key not in _NC_CACHE:
        _NC_CACHE[key] = build_nc()
    return _NC_CACHE[key]


def kernel(s, ef_mask, W, b):
    s = np.ascontiguousarray(s, dtype=np.float32)
    ef_mask = np.ascontiguousarray(ef_mask, dtype=np.float32)
    W = np.ascontiguousarray(W, dtype=np.float32)
    b = np.ascontiguousarray(b, dtype=np.float32)

    nc = _get_nc()
    in_maps = []
    for c in range(N_CORES):
        bb = c // 2
        half = c % 2
        in_maps.append(
            {
                "s": s[bb],
                "w": W,
                "b": b,
                "mask": ef_mask[bb, half * IH:(half + 1) * IH],
            }
        )
    res = run_bass_kernel_spmd(nc, in_maps, list(range(N_CORES))).results
    out = np.empty((B, N, H), dtype=np.float32)
    for c in range(N_CORES):
        bb = c // 2
        half = c % 2
        out[bb, half * IH:(half + 1) * IH] = res[c]["out"]
    return out
